# revision 33
# baseline (speedup 1.0000x reference)
"""Block-sparse flash attention on 8 TRN2 NeuronCores.

Problem: q,k,v [16, 8192, 64] fp32, block_mask [128,128] int32 (64x64 blocks).
out[h] = softmax_masked(q[h] @ k[h].T / 8) @ v[h].

Strategy (per core = 2 heads, mask shared across heads):
  - S^T layout: keys on partitions, queries on free dim.  Partitions stacked
    [64 keys of head0 | 64 keys of head1] so both heads' score blocks for key
    block j share one 128-partition tile (mask is head-independent).
  - QK matmul: block-diagonal lhsT = [[K0_j^T, 0], [0, K1_j^T]] (contract dim
    = [d h0 | d h1]), rhs = stacked Q^T, streaming only the query blocks that
    are valid for key block j (packed, host-computed from the mask).
  - exp split across ScalarE (true Exp activation, scale=1/8 fused) and the
    otherwise-idle VectorE (Schraudolph fast-exp: one fp32->int16
    tensor_scalar whose int16 result IS the bf16 bit pattern), one
    instruction per packed region, greedily balanced between the two queues
    (mask-specialized static schedule).  This halves the exp wall-time that
    previously gated both the score-PSUM reuse and the PV operand supply.
  - PV: lhsT = [V_hj | ones | pad] (ones column yields the softmax denominator
    in output row 64), rhs = the packed exp'd tile; accumulated over key
    blocks into a per-chunk O'^T PSUM tile at dense query positions
    (per-element has_written semantics make run-fragmented accumulation safe
    after a bank-clearing zero matmul).
  - Host: pre-transposes/stacks Q,K,V into bf16, divides numerator by the
    denominator row and transposes the output back.
No max-subtraction is needed: logits ~ N(0,1), exp stays tiny vs fp32 range.
"""

import os

import numpy as np
import ml_dtypes

H, N, D = 16, 8192, 64
B = 64              # mask block size
NB = N // B         # 128 blocks
NCORES = 8
HPC = H // NCORES   # heads per core
CHUNK = 1024        # queries per output chunk (PSUM-resident O' accumulator)
NCHUNK = N // CHUNK
QBC = CHUNK // B    # query blocks per chunk
REGION_W = 1024     # packed score-region width (2 PSUM banks)
SREGS = 2           # score-region PSUM ring depth (2 x 2 banks + 2 x 2 O banks)
ACT_FRAC = 0.556    # ACT engine's column share of each region's exp
BANK_W = 512        # fp32 columns per PSUM bank
VT_W = NB * 65 + 63  # V tile layout width (64 V cols + 1 ones col per block, padded)
BF16 = ml_dtypes.bfloat16

# Schraudolph fast-exp constants for the DVE share of the softmax exp:
# bf16 bits of exp(s/8) ~= int16(s * SCH_A + SCH_B); the affine runs as one
# DVE tensor_scalar (fp32 PSUM in -> int16 out) and the int16 result is the
# bf16 bit pattern (7 mantissa bits <-> 2^7 scale).  SCH_B folds the bf16
# exponent bias minus a half-ulp centering correction.
SCH_A = 0.125 * 128.0 / float(np.log(2.0))   # 23.08312...
SCH_B = 16248.9                              # bias-centered (round/trunc mid)

DEBUG = bool(int(os.environ.get("KERNEL_DEBUG", "0")))
# walrus rejects --enable-ldw-opt=true for this kernel's LDWEIGHTS shapes
# ("InstLdweights is not compatible with LDW optimization"), so it stays off.
LDW_OPT = os.environ.get("KERNEL_LDW_OPT", "0") == "1"


def _runs(sorted_ints):
    """Maximal runs of consecutive integers -> list of (start, length)."""
    out = []
    for x in sorted_ints:
        if out and x == out[-1][0] + out[-1][1]:
            out[-1][1] += 1
        else:
            out.append([x, 1])
    return [(a, b) for a, b in out]


def _plan(mask):
    """Static schedule from the mask: per chunk, key-block tiles packed into
    score regions.  Returns list over chunks of list of regions; each region is
    (tiles, W) with tiles = [(j, off, runs)], runs = [(local_qb, L)]."""
    plan = []
    for ci in range(NCHUNK):
        items = []
        for j in range(NB):
            qbs = [qb - ci * QBC for qb in range(ci * QBC, (ci + 1) * QBC)
                   if mask[qb, j]]
            if qbs:
                items.append((B * len(qbs), j, _runs(qbs)))
        # First-fit-decreasing: pack key-block tiles into score regions to
        # minimize the number of (fixed-overhead) activation instructions.
        items.sort(key=lambda x: -x[0])
        regions = []  # [ [used_cols, tiles] ]
        for w, j, runs in items:
            for reg in regions:
                if reg[0] + w <= REGION_W:
                    reg[1].append((j, reg[0], runs))
                    reg[0] += w
                    break
            else:
                regions.append([w, [(j, 0, runs)]])
        plan.append([(tiles, used) for used, tiles in regions])
    return plan


def _build_module(mask):
    import concourse.tile as tile
    from concourse import bacc, mybir

    plan = _plan(mask)

    nc = bacc.Bacc(
        "TRN2",
        debug=False,
        enable_asserts=False,
        target_bir_lowering=False,
        num_devices=NCORES,
    )
    f32 = mybir.dt.float32
    bf16 = mybir.dt.bfloat16
    Exp = mybir.ActivationFunctionType.Exp

    q2t = nc.dram_tensor("q2t", [128, N], bf16, kind="ExternalInput").ap()
    k2t = nc.dram_tensor("k2t", [128, NB * 128], bf16, kind="ExternalInput").ap()
    vt = nc.dram_tensor("vt", [128, VT_W], bf16, kind="ExternalInput").ap()
    out = nc.dram_tensor("out", [HPC, 65, N], f32, kind="ExternalOutput").ap()

    with tile.TileContext(nc) as tc:
        with (
            tc.tile_pool(name="res", bufs=1) as res,
            tc.tile_pool(name="psum", bufs=1, space="PSUM") as psum,
            tc.tile_pool(name="pbuf", bufs=1) as pbuf,
            tc.tile_pool(name="stage", bufs=2) as stage_pool,
        ):
            q2sb = res.tile([128, N], bf16, tag="q2sb", name="q2sb")
            k2sb = res.tile([128, NB * 128], bf16, tag="k2sb", name="k2sb")
            vtsb = res.tile([128, VT_W], bf16, tag="vtsb", name="vtsb")
            # Split resident loads and order them by first use in the static
            # schedule, so the first regions' matmuls start as early as
            # possible instead of waiting for the whole working set.
            use_pos = {}
            pos = 0
            for ci in range(NCHUNK):
                for tiles, _w in plan[ci]:
                    for (j, _off, _r) in tiles:
                        use_pos.setdefault(j, pos)
                        pos += 1
            KG = 4   # k2 piece = 4 key blocks
            kg_order = sorted(range(NB // KG),
                              key=lambda g: min(use_pos.get(j, 1 << 30)
                                                for j in range(g * KG, (g + 1) * KG)))
            VG = 8   # vt piece = 8 key blocks
            vg_order = sorted(range(NB // VG),
                              key=lambda g: min(use_pos.get(j, 1 << 30)
                                                for j in range(g * VG, (g + 1) * VG)))
            nc.sync.dma_start(out=q2sb[:, 0:CHUNK], in_=q2t[:, 0:CHUNK])
            for i in range(max(len(kg_order), len(vg_order) + 1)):
                if i < len(kg_order):
                    g = kg_order[i]
                    lo, hi = g * KG * 128, (g + 1) * KG * 128
                    nc.sync.dma_start(out=k2sb[:, lo:hi], in_=k2t[:, lo:hi])
                if 0 < i <= len(vg_order):
                    g = vg_order[i - 1]
                    lo = g * VG * 65
                    hi = VT_W if g == NB // VG - 1 else (g + 1) * VG * 65
                    nc.sync.dma_start(out=vtsb[:, lo:hi], in_=vt[:, lo:hi])
            for p in range(1, NCHUNK):
                nc.sync.dma_start(
                    out=q2sb[:, p * CHUNK:(p + 1) * CHUNK],
                    in_=q2t[:, p * CHUNK:(p + 1) * CHUNK],
                )

            o_ps = [
                psum.tile([128, CHUNK], f32, tag="o0", name="o0"),
                psum.tile([128, CHUNK], f32, tag="o1", name="o1"),
            ]
            s_ps = [
                psum.tile([128, REGION_W], f32, tag=f"s{i}", name=f"s{i}")
                for i in range(SREGS)
            ]
            p_sb = [
                pbuf.tile([128, REGION_W], bf16, tag=f"p{i}", name=f"p{i}")
                for i in range(6)
            ]

            def emit_qk(ci, tiles, rb):
                S = s_ps[rb]
                for (j, off, runs) in tiles:
                    lhs = k2sb[:, j * 128:(j + 1) * 128]
                    local = off
                    for (q0, L) in runs:
                        seg = B * L
                        s = 0
                        while s < seg:
                            e = min(seg, ((local + s) // BANK_W + 1) * BANK_W - local)
                            rq = ci * CHUNK + q0 * B + s
                            nc.tensor.matmul(
                                S[:, local + s:local + e],
                                lhsT=lhs,
                                rhs=q2sb[:, rq:rq + (e - s)],
                                start=True, stop=True,
                                skip_group_check=True,
                            )
                            s = e
                        local += seg

            # The exp is the second bottleneck (ScalarE ~94% busy in the
            # baseline): split EACH region's columns between the ACT engine
            # (true Exp) and the otherwise-idle DVE (Schraudolph fast-exp:
            # one fp32->int16 tensor_scalar whose int16 result IS the bf16
            # bit pattern).  Running both halves in parallel keeps per-region
            # exp latency under the PE's per-region streaming time, so the
            # 2-deep score-PSUM ring never stalls the QK matmuls.
            eng_load = {"act": 0.0, "dve": 0.0}
            mult = mybir.AluOpType.mult
            add = mybir.AluOpType.add

            def emit_act(rb, pb, W):
                t_act = (W + 204) * (1.0 / 1.2)
                t_dve = (W + 163) * (1.0 / 0.96)
                if eng_load["act"] + t_act <= eng_load["dve"] + t_dve:
                    eng_load["act"] += t_act
                    nc.scalar.activation(
                        p_sb[pb][:, 0:W], s_ps[rb][:, 0:W], Exp, scale=0.125
                    )
                else:
                    eng_load["dve"] += t_dve
                    nc.vector.tensor_scalar(
                        p_sb[pb][:, 0:W].bitcast(mybir.dt.int16),
                        s_ps[rb][:, 0:W],
                        SCH_A,
                        SCH_B,
                        mult,
                        add,
                    )

            # The first PV matmul touching each O' PSUM bank per chunk uses
            # start=True: it clears the whole bank's has_written bits and
            # overwrites its own columns; every later matmul (start=False)
            # overwrites where the bit is clear and accumulates where set.
            # This removes the explicit bank-zeroing matmuls entirely.
            ft = {"ci": None, "seen": set()}

            def emit_pv(ci, tiles, pb, is_last_of_chunk):
                if ft["ci"] != ci:
                    ft["ci"] = ci
                    ft["seen"] = set()
                P = p_sb[pb]
                for ti, (j, off, runs) in enumerate(tiles):
                    local = off
                    for ri, (q0, L) in enumerate(runs):
                        seg = B * L
                        c0 = q0 * B
                        s = 0
                        while s < seg:
                            e = min(seg, ((c0 + s) // BANK_W + 1) * BANK_W - c0)
                            last = (
                                is_last_of_chunk
                                and ti == len(tiles) - 1
                                and ri == len(runs) - 1
                                and e == seg
                            )
                            # Alternate heads per piece: adjacent matmuls use
                            # disjoint PE row groups, enabling LDWEIGHTS
                            # pull-ahead and fill/drain overlap between them.
                            for h in (0, 1):
                                bank = (h, (c0 + s) // BANK_W)
                                first = bank not in ft["seen"]
                                if first:
                                    ft["seen"].add(bank)
                                nc.tensor.matmul(
                                    o_ps[h][:, c0 + s:c0 + e],
                                    lhsT=vtsb[h * 64:(h + 1) * 64,
                                              j * 65:j * 65 + 128],
                                    rhs=P[h * 64:(h + 1) * 64,
                                          local + s:local + e],
                                    start=first, stop=last,
                                    skip_group_check=True,
                                )
                            s = e
                        local += seg

            def emit_epilogue(ci):
                # Split the two evacuation copies across ScalarE and VectorE so
                # they run in parallel at the chunk boundary (DMA cannot read
                # PSUM directly on this target).
                for h in (0, 1):
                    st = stage_pool.tile(
                        [65, CHUNK], f32, tag=f"st{h}", name=f"st{h}_{ci}"
                    )
                    if h == 0:
                        nc.scalar.copy(st[:], o_ps[h][0:65, :])
                    else:
                        nc.vector.tensor_copy(st[:], o_ps[h][0:65, :])
                    nc.sync.dma_start(
                        out=out[h, :, ci * CHUNK:(ci + 1) * CHUNK], in_=st[:]
                    )

            flat = []
            for ci in range(NCHUNK):
                nreg = len(plan[ci])
                for ri, (tiles, W) in enumerate(plan[ci]):
                    flat.append((ci, tiles, W, ri == 0, ri == nreg - 1))

            # Warm the PE HAM clock gate during the initial DMA wait with
            # garbage matmuls on the first q2 piece (the earliest-landing DMA;
            # outputs are overwritten by the real schedule).
            for w in range(8):
                nc.tensor.matmul(
                    s_ps[1][:, (w % 2) * BANK_W:(w % 2 + 1) * BANK_W],
                    lhsT=q2sb[:, 0:128],
                    rhs=q2sb[:, 0:BANK_W],
                    start=True, stop=True, skip_group_check=True,
                )

            # Software-pipeline with the PV pass lagging FOUR regions behind
            # QK/exp, so each region's exp (on either engine) has several
            # QK-regions of cover before its PV consumers run (6-deep P ring);
            # the 3-deep score ring gives QK two regions of slack on the exp.
            pending = []

            def flush_one():
                pci, ptiles, ppb, pfirst, plast = pending.pop(0)
                emit_pv(pci, ptiles, ppb, plast)
                if plast:
                    emit_epilogue(pci)

            for gi, (ci, tiles, W, first, last) in enumerate(flat):
                rb = gi % SREGS
                pb = gi % 6
                emit_qk(ci, tiles, rb)
                emit_act(rb, pb, W)
                pending.append((ci, tiles, pb, first, last))
                if len(pending) > 4:
                    flush_one()
            while pending:
                flush_one()

    nc.finalize()

    if DEBUG:
        tot_w = sum(W for regs in plan for (_, W) in regs)
        nregs = sum(len(regs) for regs in plan)
        print(f"[build] regions={nregs} total packed cols={tot_w} "
              f"exp split act={eng_load['act'] / 1e3:.1f}us "
              f"dve={eng_load['dve'] / 1e3:.1f}us")
    return nc


def _prep_core(qf, kf, vf, h0, h1):
    q2t = np.empty((128, N), BF16)
    q2t[:64] = qf[h0].T
    q2t[64:] = qf[h1].T

    k2t = np.zeros((128, NB * 128), BF16)
    k2t[:64].reshape(64, NB, 128)[:, :, :64] = (
        kf[h0].T.astype(BF16).reshape(64, NB, 64)
    )
    k2t[64:].reshape(64, NB, 128)[:, :, 64:] = (
        kf[h1].T.astype(BF16).reshape(64, NB, 64)
    )

    vt = np.zeros((128, VT_W), BF16)
    vt[:64, :NB * 65].reshape(64, NB, 65)[:, :, :64] = (
        vf[h0].reshape(NB, 64, 64).transpose(1, 0, 2).astype(BF16)
    )
    vt[:64, :NB * 65].reshape(64, NB, 65)[:, :, 64] = 1.0
    vt[64:, :NB * 65].reshape(64, NB, 65)[:, :, :64] = (
        vf[h1].reshape(NB, 64, 64).transpose(1, 0, 2).astype(BF16)
    )
    vt[64:, :NB * 65].reshape(64, NB, 65)[:, :, 64] = 1.0
    return {"q2t": q2t, "k2t": k2t, "vt": vt}


def kernel(q, k, v, block_mask):
    import concourse.bass_utils as bu
    from concourse.bass_utils import run_bass_kernel_spmd

    qf = np.asarray(q, dtype=np.float32)
    kf = np.asarray(k, dtype=np.float32)
    vf = np.asarray(v, dtype=np.float32)
    mask = np.asarray(block_mask) != 0

    nc = _build_module(mask)
    in_maps = [_prep_core(qf, kf, vf, 2 * c, 2 * c + 1) for c in range(NCORES)]

    # Redundant-LDWEIGHTS elision in walrus codegen: consecutive matmuls in a
    # packed region share the same stationary operand, and the default
    # (--enable-ldw-opt=false) re-loads it before every matmul.
    orig_run_command = bu.run_command

    def _run_command_ldw(argv, **kw):
        argv = [
            a.replace("--enable-ldw-opt=false", "--enable-ldw-opt=true")
            for a in argv
        ]
        return orig_run_command(argv, **kw)

    if LDW_OPT:
        bu.run_command = _run_command_ldw
    try:
        res = run_bass_kernel_spmd(nc, in_maps, core_ids=list(range(NCORES)))
    finally:
        bu.run_command = orig_run_command

    o_full = np.empty((H, N, D), dtype=np.float32)
    empty_rows = np.repeat(mask.sum(axis=1) == 0, B)
    for c in range(NCORES):
        ot = res.results[c]["out"]  # [2, 65, N]
        with np.errstate(divide="ignore", invalid="ignore"):
            o = ot[:, :64, :] / ot[:, 64:65, :]
        o_full[2 * c:2 * c + 2] = o.transpose(0, 2, 1)
    if empty_rows.any():
        o_full[:, empty_rows, :] = np.nan
    return o_full

